# revision 16
# baseline (speedup 1.0000x reference)
"""DeltaRuleGated Trainium2 kernel (v5).

Recurrence per (b,h) pair over T time steps, state M[128,128]:
    M_t = M_{t-1} * max(f_t (x) f_t, 0.8) + (k_t*g_t) (x) (v_t*g_t)
    o_t = q_t^T M_t
(upper clip at 1.0 is a no-op: f in [0,1) so f_d*f_e < 1)

Sharding: 32 (b,h) pairs -> 8 cores x 4 pairs, no cross-core comm.

Per-core design (v5: all-bf16 PE, merged banks, temporal double-buffer):
  - All outer products on PE as bf16 matmuls. Numerics: bf16 rounding of
    f/u/w adds ~1e-3 rel err on top of the bf16-state ~4e-3; validated
    against fp64 in numpy (total ~4.6e-3, tolerance 2e-2).
  - ONE F matmul per step: K=4 block-diag (4 pairs), N=512, into a full
    PSUM bank; ONE delta matmul likewise. Banks are double-buffered by
    step parity so the PE never waits on DVE/ACT readers (WAR).
  - stat rows 0..3: f per pair (slot order), rows 32..35: u = k*g.
    strm rows 0..3: f block-diag at [512t + 128s + e]; rows 32..35: w.
  - DVE per step: A = scalar_tensor_tensor max(bankF,0.8)*M -> mp (bf16)
    and B = mp + dsb (bf16 2x). ACT evacuates bankD -> dsb bf16.
  - Output: masked-Q matvec, ONE matmul per step: lhsT = Q4_j
    [128,128] bf16, zero except col 32*slot(p)+j = q_{p,t0+j};
    rhs = m_new [128,512]; accumulates in PSUM bankO where row
    32*slot(p)+j of pair p's block is o_{p,t0+j}. One evacuation per C.
"""

import numpy as np

import concourse.bass as bass
import concourse.bacc as bacc
import concourse.tile as tile
from concourse import mybir
from concourse.bass_utils import run_bass_kernel_spmd

B, T, H, D = 4, 2048, 8, 128
N_CORES = 8
NP = (B * H) // N_CORES  # pairs per core = 4
C = 32                   # time steps per chunk (= output group size)
F32 = mybir.dt.float32
F32R = mybir.dt.float32r
BF16 = mybir.dt.bfloat16
AOP = mybir.AluOpType
AF = mybir.ActivationFunctionType
PSUM = bass.MemorySpace.PSUM

EVOD = [0, 2, 1, 3]                       # state slot order
IDX = {p: i for i, p in enumerate(EVOD)}  # pair -> slot


def build(t_run=T):
    nch = t_run // C
    CD = C * D
    nc = bacc.Bacc(None, target_bir_lowering=False)

    dqt = nc.dram_tensor("qt", [NP, D, t_run], BF16, kind="ExternalInput")
    dk = nc.dram_tensor("k", [NP, t_run, D], F32, kind="ExternalInput")
    dv = nc.dram_tensor("v", [NP, t_run, D], F32, kind="ExternalInput")
    df = nc.dram_tensor("f", [NP, t_run, D], BF16, kind="ExternalInput")
    dg = nc.dram_tensor("g", [NP, t_run, D], F32, kind="ExternalInput")
    dzero = nc.dram_tensor("zeros", [C, D], F32R, kind="ExternalInput")
    dout = nc.dram_tensor("out", [NP, t_run, D], F32, kind="ExternalOutput")

    with tile.TileContext(nc) as tc:
        with (
            tc.tile_pool(name="singles", bufs=1) as singles,
            tc.tile_pool(name="stage", bufs=2) as stage,
            tc.tile_pool(name="prep", bufs=2) as prep,
            tc.tile_pool(name="state", bufs=4) as statep,
            tc.tile_pool(name="step", bufs=5) as stepp,
            tc.tile_pool(name="outp", bufs=2) as outp,
            tc.tile_pool(name="psF", bufs=1, space=PSUM) as psF,
            tc.tile_pool(name="psD", bufs=1, space=PSUM) as psD,
            tc.tile_pool(name="psO", bufs=2, space=PSUM) as psO,
        ):
            # Q4 regions (x2, alternating by chunk parity): [128, C*129]
            # bf16. Step tile j = flat cols [128j, 128j+128); pair p's q
            # column lands at flat col 129j + 32*slot(p) = local col
            # 32*slot(p)+j of tile_j. Other cols stay zero forever.
            q4rs = [
                singles.tile([D, C * (D + 1)], BF16, name=f"q4r{i}", tag=f"q4r{i}")
                for i in range(2)
            ]
            for i in range(2):
                # zero-fill via broadcast DMA (gpsimd memset of this much
                # SBUF takes ~10us; the DMA is ~1us, once)
                z = q4rs[i].bitcast(F32R)
                nc.sync.dma_start(
                    out=z[:, :],
                    in_=bass.AP(
                        tensor=dzero, offset=0,
                        ap=[[0, D], [1, z.shape[1]]],
                    ),
                )

            # weight tiles x2 (alternating by chunk parity so a chunk's
            # weight loads overlap the previous chunk's compute); zero
            # gaps in strm memset once per buffer.
            stats = [
                singles.tile([36, CD], BF16, name=f"stat{i}", tag=f"stat{i}")
                for i in range(2)
            ]
            strms = [
                singles.tile([36, 4 * CD], BF16, name=f"strm{i}", tag=f"strm{i}")
                for i in range(2)
            ]
            for i in range(2):
                zv = strms[i].bitcast(F32R)  # [36, 2*CD] f32-sized view
                for r in (0, 32):
                    nc.sync.dma_start(
                        out=zv[r : r + 4, :],
                        in_=bass.AP(
                            tensor=dzero, offset=0,
                            ap=[[0, 4], [0, 2], [1, C * D]],
                        ),
                    )

            # persistent PSUM banks, triple-buffered by global step index
            # mod 3 (one tile_position each, hw requirement per bank):
            # F(t+3) -> bank[t%3] only has a WAR dependency on A(t), which
            # the DVE finished long before the PE gets there. 3F+3D+2O = 8.
            bankF = [psF.tile([D, NP * D], F32, name=f"bankF{i}", tag=f"f{i}")
                     for i in range(3)]
            bankD = [psD.tile([D, NP * D], F32, name=f"bankD{i}", tag=f"d{i}")
                     for i in range(3)]

            # initial state M = 0 (bf16, slot order EVOD)
            m_prev = statep.tile([D, NP * D], BF16, tag="M")
            nc.gpsimd.memset(m_prev[:, :], 0.0)

            evac_prev = [None]

            def emit_output(oS_prev, t0_prev):
                # pair p's outputs: rows [32*slot, 32*slot+C) of its block.
                # Emitted AFTER the next chunk's load DMAs so the out DMAs
                # (which wait on that chunk's last matvec) land on the
                # in-order HW DMA rings BEHIND the loads — otherwise they
                # head-of-line block the next chunk's weight prefetch for
                # the whole previous chunk tail (~6us/chunk).
                for p in range(NP):
                    sl = IDX[p]
                    nc.scalar.dma_start(
                        out=dout[p, t0_prev : t0_prev + C, :],
                        in_=oS_prev[32 * sl : 32 * sl + C, sl * D : (sl + 1) * D],
                    )

            pending_out = None
            for ch in range(nch):
                t0 = ch * C
                stat = stats[ch % 2]
                strm = strms[ch % 2]
                q4r = q4rs[ch % 2]
                # ---- staging (k, g, v) : [C, NP, D] f32
                kS = stage.tile([C, NP, D], F32, tag="kS")
                vS = stage.tile([C, NP, D], F32, tag="vS")
                gS = stage.tile([C, NP, D], F32, tag="gS")
                for dst, src in ((kS, dk), (vS, dv), (gS, dg)):
                    nc.sync.dma_start(
                        out=dst[:, :, :],
                        in_=src[:, t0 : t0 + C, :].rearrange("p t d -> t p d"),
                    )

                uF = prep.tile([C, NP, D], BF16, tag="uF")
                wF = prep.tile([C, NP, D], BF16, tag="wF")
                nc.gpsimd.tensor_mul(uF[:, :, :], kS[:, :, :], gS[:, :, :])
                nc.gpsimd.tensor_mul(wF[:, :, :], vS[:, :, :], gS[:, :, :])

                # ---- weight loads
                # stat rows: {0..3}: f per slot   {32..35}: u per slot
                # strm rows: same indices; block-diag: slot s holds its
                #   sequence at free [512t + 128s, +128).
                for p in range(NP):
                    s = IDX[p]
                    # f (stationary packed + stream block-diag) from HBM;
                    # emitted before the prep-dependent u/w DMAs so the
                    # in-order SP queue never head-blocks on POOL prep.
                    nc.sync.dma_start(
                        out=stat[s : s + 1, :],
                        in_=df[p, t0 : t0 + C, :],
                    )
                    nc.sync.dma_start(
                        out=strm[s : s + 1, :].rearrange(
                            "o (t b d) -> o t b d", b=NP, d=D
                        )[:, :, s, :],
                        in_=df[p, t0 : t0 + C, :],
                    )
                for p in range(NP):
                    s = IDX[p]
                    nc.sync.dma_start(
                        out=stat[32 + s : 33 + s, :],
                        in_=uF[:, p, :],
                    )
                    nc.sync.dma_start(
                        out=strm[32 + s : 33 + s, :].rearrange(
                            "o (t b d) -> o t b d", b=NP, d=D
                        )[:, :, s, :],
                        in_=wF[:, p, :],
                    )

                # ---- q (host-pretransposed) -> scatter into Q4 region
                for p in range(NP):
                    qT = stepp.tile([D, C, 1], BF16, tag="qT", name="qT")
                    nc.sync.dma_start(
                        out=qT[:, :, 0], in_=dqt[p, :, t0 : t0 + C]
                    )
                    qv = q4r.rearrange("a (j c) -> a j c", c=D + 1)
                    sl = 32 * IDX[p]
                    nc.gpsimd.tensor_copy(qv[:, :, sl : sl + 1], qT[:, :, 0:1])

                # previous chunk's output DMAs go after this chunk's loads
                if pending_out is not None:
                    emit_output(*pending_out)
                    pending_out = None

                oS = outp.tile([D, NP * D], F32, tag="oS")
                bankO = psO.tile([D, NP * D], F32, tag="bankO")

                # ---- sequential scan over this chunk's steps.
                # Outer products + bankD evacuation for step j are emitted
                # during step j-1 (software pipelining): keeps the in-order
                # PE queue from stalling next-step matmuls behind the
                # current matvec, and keeps ACT evacuations in step order.
                def emit_outers(j):
                    js = slice(j * D, (j + 1) * D)
                    j4 = slice(j * 4 * D, (j + 1) * 4 * D)
                    bF = bankF[(t0 + j) % 3]
                    bD = bankD[(t0 + j) % 3]
                    nc.tensor.matmul(
                        bF[:, :], stat[0:4, js], strm[0:4, j4],
                        start=True, stop=True, tile_position=(0, 0),
                    )
                    nc.tensor.matmul(
                        bD[:, :], stat[32:36, js], strm[32:36, j4],
                        start=True, stop=True, tile_position=(32, 0),
                    )
                    dsb = stepp.tile([D, NP * D], BF16, tag="dsb", name="dsb")
                    e1 = nc.scalar.activation(dsb[:, :], bD[:, :], AF.Copy)
                    # keep ACT in step order (scheduler otherwise scrambles)
                    if evac_prev[0] is not None:
                        tile.add_dep_helper(e1.ins, evac_prev[0].ins, False, "ACT order")
                    evac_prev[0] = e1
                    return dsb

                # lookahead-3 software pipeline: PE queue per step is
                # [F(j+3), D(j+3), matvec(j)] so when matvec(j) waits on
                # B(j), the PE has already streamed three steps of outers
                # and (with no gaps) can ramp to its max pstate.
                dsb_q = [emit_outers(0), emit_outers(1), emit_outers(2)]
                for j in range(C):
                    dsb = dsb_q.pop(0)
                    mp = stepp.tile([D, NP * D], BF16, tag="mp")
                    m_new = statep.tile([D, NP * D], BF16, tag="M")
                    nc.vector.scalar_tensor_tensor(
                        out=mp[:, :], in0=bankF[(t0 + j) % 3][:, :], scalar=0.8,
                        in1=m_prev[:, :], op0=AOP.max, op1=AOP.mult,
                    )
                    nc.vector.tensor_add(m_new[:, :], mp[:, :], dsb[:, :])

                    if j + 3 < C:
                        dsb_q.append(emit_outers(j + 3))

                    # masked-Q matvec: one matmul, all pairs
                    nc.tensor.matmul(
                        bankO[:, :],
                        q4r[:, j * D : (j + 1) * D],
                        m_new[:, :],
                        start=(j == 0), stop=(j == C - 1),
                        tile_position=(0, 0),
                    )
                    m_prev = m_new

                nc.scalar.activation(oS[:, :], bankO[:, :], AF.Copy)
                pending_out = (oS, t0)

            if pending_out is not None:
                emit_output(*pending_out)

    nc.compile()
    return nc


_CACHE = {}


def _get_nc(t_run):
    if t_run not in _CACHE:
        _CACHE[t_run] = build(t_run)
    return _CACHE[t_run]


def kernel(q, k, v, f_gate, g_gate):
    t_run = q.shape[1]
    nc = _get_nc(t_run)

    def shard(x):
        # [B, T, H, D] -> [B*H, T, D] -> per-core [NP, T, D]
        xt = np.ascontiguousarray(
            np.transpose(np.asarray(x, dtype=np.float32), (0, 2, 1, 3))
        ).reshape(B * H, t_run, D)
        return [np.ascontiguousarray(xt[c * NP : (c + 1) * NP]) for c in range(N_CORES)]

    qs, ks, vs, fs, gs = (shard(x) for x in (q, k, v, f_gate, g_gate))
    import ml_dtypes
    qts = [
        np.ascontiguousarray(np.transpose(x, (0, 2, 1))).astype(ml_dtypes.bfloat16)
        for x in qs
    ]
    fs = [x.astype(ml_dtypes.bfloat16) for x in fs]
    zeros = np.zeros((C, D), dtype=np.float32)
    in_maps = [
        {"qt": qts[c], "k": ks[c], "v": vs[c], "f": fs[c], "g": gs[c],
         "zeros": zeros}
        for c in range(N_CORES)
    ]
    global _LAST_NC, _LAST_IN_MAPS
    _LAST_NC, _LAST_IN_MAPS = nc, in_maps
    res = run_bass_kernel_spmd(nc, in_maps, core_ids=list(range(N_CORES)))
    full = np.concatenate([res.results[c]["out"] for c in range(N_CORES)], axis=0)
    # [B*H, T, D] -> [B, T, H, D]
    return np.ascontiguousarray(
        np.transpose(full.reshape(B, H, t_run, D), (0, 2, 1, 3))
    )


# revision 17
# speedup vs baseline: 1.0767x; 1.0767x over previous
"""DeltaRuleGated Trainium2 kernel (v5).

Recurrence per (b,h) pair over T time steps, state M[128,128]:
    M_t = M_{t-1} * max(f_t (x) f_t, 0.8) + (k_t*g_t) (x) (v_t*g_t)
    o_t = q_t^T M_t
(upper clip at 1.0 is a no-op: f in [0,1) so f_d*f_e < 1)

Sharding: 32 (b,h) pairs -> 8 cores x 4 pairs, no cross-core comm.

Per-core design (v5: all-bf16 PE, merged banks, temporal double-buffer):
  - All outer products on PE as bf16 matmuls. Numerics: bf16 rounding of
    f/u/w adds ~1e-3 rel err on top of the bf16-state ~4e-3; validated
    against fp64 in numpy (total ~4.6e-3, tolerance 2e-2).
  - ONE F matmul per step: K=4 block-diag (4 pairs), N=512, into a full
    PSUM bank; ONE delta matmul likewise. Banks are double-buffered by
    step parity so the PE never waits on DVE/ACT readers (WAR).
  - stat rows 0..3: f per pair (slot order), rows 32..35: u = k*g.
    strm rows 0..3: f block-diag at [512t + 128s + e]; rows 32..35: w.
  - DVE per step: A = scalar_tensor_tensor max(bankF,0.8)*M -> mp (bf16)
    and B = mp + dsb (bf16 2x). ACT evacuates bankD -> dsb bf16.
  - Output: masked-Q matvec, ONE matmul per step: lhsT = Q4_j
    [128,128] bf16, zero except col 32*slot(p)+j = q_{p,t0+j};
    rhs = m_new [128,512]; accumulates in PSUM bankO where row
    32*slot(p)+j of pair p's block is o_{p,t0+j}. One evacuation per C.
"""

import numpy as np

import concourse.bass as bass
import concourse.bacc as bacc
import concourse.tile as tile
from concourse import mybir
from concourse.bass_utils import run_bass_kernel_spmd

B, T, H, D = 4, 2048, 8, 128
N_CORES = 8
NP = (B * H) // N_CORES  # pairs per core = 4
C = 32                   # time steps per chunk (= output group size)
F32 = mybir.dt.float32
F32R = mybir.dt.float32r
BF16 = mybir.dt.bfloat16
AOP = mybir.AluOpType
AF = mybir.ActivationFunctionType
PSUM = bass.MemorySpace.PSUM

EVOD = [0, 2, 1, 3]                       # state slot order
IDX = {p: i for i, p in enumerate(EVOD)}  # pair -> slot


def build(t_run=T):
    nch = t_run // C
    CD = C * D
    nc = bacc.Bacc(None, target_bir_lowering=False)

    dqt = nc.dram_tensor("qt", [NP, D, t_run], BF16, kind="ExternalInput")
    dk = nc.dram_tensor("k", [NP, t_run, D], F32, kind="ExternalInput")
    dv = nc.dram_tensor("v", [NP, t_run, D], F32, kind="ExternalInput")
    df = nc.dram_tensor("f", [NP, t_run, D], BF16, kind="ExternalInput")
    dg = nc.dram_tensor("g", [NP, t_run, D], F32, kind="ExternalInput")
    dzero = nc.dram_tensor("zeros", [C, D], F32R, kind="ExternalInput")
    dout = nc.dram_tensor("out", [NP, t_run, D], F32, kind="ExternalOutput")

    with tile.TileContext(nc) as tc:
        with (
            tc.tile_pool(name="singles", bufs=1) as singles,
            tc.tile_pool(name="stage", bufs=2) as stage,
            tc.tile_pool(name="prep", bufs=2) as prep,
            tc.tile_pool(name="state", bufs=4) as statep,
            tc.tile_pool(name="step", bufs=5) as stepp,
            tc.tile_pool(name="outp", bufs=2) as outp,
            tc.tile_pool(name="psF", bufs=1, space=PSUM) as psF,
            tc.tile_pool(name="psD", bufs=1, space=PSUM) as psD,
            tc.tile_pool(name="psO", bufs=2, space=PSUM) as psO,
        ):
            # Q4 regions (x2, alternating by chunk parity): [128, C*129]
            # bf16. Step tile j = flat cols [128j, 128j+128); pair p's q
            # column lands at flat col 129j + 32*slot(p) = local col
            # 32*slot(p)+j of tile_j. Other cols stay zero forever.
            q4rs = [
                singles.tile([D, C * (D + 1)], BF16, name=f"q4r{i}", tag=f"q4r{i}")
                for i in range(2)
            ]
            for i in range(2):
                # zero-fill via broadcast DMA (gpsimd memset of this much
                # SBUF takes ~10us; the DMA is ~1us, once)
                z = q4rs[i].bitcast(F32R)
                nc.sync.dma_start(
                    out=z[:, :],
                    in_=bass.AP(
                        tensor=dzero, offset=0,
                        ap=[[0, D], [1, z.shape[1]]],
                    ),
                )

            # weight tiles x2 (alternating by chunk parity so a chunk's
            # weight loads overlap the previous chunk's compute); zero
            # gaps in strm memset once per buffer.
            stats = [
                singles.tile([36, CD], BF16, name=f"stat{i}", tag=f"stat{i}")
                for i in range(2)
            ]
            strms = [
                singles.tile([36, 4 * CD], BF16, name=f"strm{i}", tag=f"strm{i}")
                for i in range(2)
            ]
            for i in range(2):
                zv = strms[i].bitcast(F32R)  # [36, 2*CD] f32-sized view
                for r in (0, 32):
                    nc.sync.dma_start(
                        out=zv[r : r + 4, :],
                        in_=bass.AP(
                            tensor=dzero, offset=0,
                            ap=[[0, 4], [0, 2], [1, C * D]],
                        ),
                    )

            # persistent PSUM banks, triple-buffered by global step index
            # mod 3 (one tile_position each, hw requirement per bank):
            # F(t+3) -> bank[t%3] only has a WAR dependency on A(t), which
            # the DVE finished long before the PE gets there. 3F+3D+2O = 8.
            bankF = [psF.tile([D, NP * D], F32, name=f"bankF{i}", tag=f"f{i}")
                     for i in range(3)]
            bankD = [psD.tile([D, NP * D], F32, name=f"bankD{i}", tag=f"d{i}")
                     for i in range(3)]

            # initial state M = 0 (bf16, slot order EVOD)
            m_prev = statep.tile([D, NP * D], BF16, tag="M")
            nc.gpsimd.memset(m_prev[:, :], 0.0)

            evac_prev = [None]

            def emit_output(oS_prev, t0_prev):
                # pair p's outputs: rows [32*slot, 32*slot+C) of its block.
                # Emitted AFTER the next chunk's load DMAs so the out DMAs
                # (which wait on that chunk's last matvec) land on the
                # in-order HW DMA rings BEHIND the loads — otherwise they
                # head-of-line block the next chunk's weight prefetch for
                # the whole previous chunk tail (~6us/chunk).
                for p in range(NP):
                    sl = IDX[p]
                    nc.scalar.dma_start(
                        out=dout[p, t0_prev : t0_prev + C, :],
                        in_=oS_prev[32 * sl : 32 * sl + C, sl * D : (sl + 1) * D],
                    )

            pending_out = None
            for ch in range(nch):
                t0 = ch * C
                stat = stats[ch % 2]
                strm = strms[ch % 2]
                q4r = q4rs[ch % 2]
                # ---- staging (k, g, v) : [C, NP, D] f32
                kS = stage.tile([C, NP, D], F32, tag="kS")
                vS = stage.tile([C, NP, D], F32, tag="vS")
                gS = stage.tile([C, NP, D], F32, tag="gS")
                for dst, src in ((kS, dk), (vS, dv), (gS, dg)):
                    nc.sync.dma_start(
                        out=dst[:, :, :],
                        in_=src[:, t0 : t0 + C, :].rearrange("p t d -> t p d"),
                    )

                uF = prep.tile([C, NP, D], BF16, tag="uF")
                wF = prep.tile([C, NP, D], BF16, tag="wF")
                nc.gpsimd.tensor_mul(uF[:, :, :], kS[:, :, :], gS[:, :, :])
                nc.gpsimd.tensor_mul(wF[:, :, :], vS[:, :, :], gS[:, :, :])

                # ---- weight loads
                # stat rows: {0..3}: f per slot   {32..35}: u per slot
                # strm rows: same indices; block-diag: slot s holds its
                #   sequence at free [512t + 128s, +128).
                for p in range(NP):
                    s = IDX[p]
                    # f (stationary packed + stream block-diag) from HBM;
                    # emitted before the prep-dependent u/w DMAs so the
                    # in-order SP queue never head-blocks on POOL prep.
                    nc.sync.dma_start(
                        out=stat[s : s + 1, :],
                        in_=df[p, t0 : t0 + C, :],
                    )
                    nc.sync.dma_start(
                        out=strm[s : s + 1, :].rearrange(
                            "o (t b d) -> o t b d", b=NP, d=D
                        )[:, :, s, :],
                        in_=df[p, t0 : t0 + C, :],
                    )
                # u/w loads issue from the Pool engine's SWDGE: they depend
                # on the Pool muls above, and putting them on SP's in-order
                # queue head-blocks ALL later SP DMAs (next chunk's staging
                # and f prefetch) until this chunk's prep finishes —
                # lag-locking the whole load pipeline one chunk behind.
                for p in range(NP):
                    s = IDX[p]
                    nc.gpsimd.dma_start(
                        out=stat[32 + s : 33 + s, :],
                        in_=uF[:, p, :],
                    )
                    nc.gpsimd.dma_start(
                        out=strm[32 + s : 33 + s, :].rearrange(
                            "o (t b d) -> o t b d", b=NP, d=D
                        )[:, :, s, :],
                        in_=wF[:, p, :],
                    )

                # ---- q (host-pretransposed) -> scatter into Q4 region
                for p in range(NP):
                    qT = stepp.tile([D, C, 1], BF16, tag="qT", name="qT")
                    nc.sync.dma_start(
                        out=qT[:, :, 0], in_=dqt[p, :, t0 : t0 + C]
                    )
                    qv = q4r.rearrange("a (j c) -> a j c", c=D + 1)
                    sl = 32 * IDX[p]
                    nc.gpsimd.tensor_copy(qv[:, :, sl : sl + 1], qT[:, :, 0:1])

                # previous chunk's output DMAs go after this chunk's loads
                if pending_out is not None:
                    emit_output(*pending_out)
                    pending_out = None

                oS = outp.tile([D, NP * D], F32, tag="oS")
                bankO = psO.tile([D, NP * D], F32, tag="bankO")

                # ---- sequential scan over this chunk's steps.
                # Outer products + bankD evacuation for step j are emitted
                # during step j-1 (software pipelining): keeps the in-order
                # PE queue from stalling next-step matmuls behind the
                # current matvec, and keeps ACT evacuations in step order.
                def emit_outers(j):
                    js = slice(j * D, (j + 1) * D)
                    j4 = slice(j * 4 * D, (j + 1) * 4 * D)
                    bF = bankF[(t0 + j) % 3]
                    bD = bankD[(t0 + j) % 3]
                    nc.tensor.matmul(
                        bF[:, :], stat[0:4, js], strm[0:4, j4],
                        start=True, stop=True, tile_position=(0, 0),
                    )
                    nc.tensor.matmul(
                        bD[:, :], stat[32:36, js], strm[32:36, j4],
                        start=True, stop=True, tile_position=(32, 0),
                    )
                    dsb = stepp.tile([D, NP * D], BF16, tag="dsb", name="dsb")
                    e1 = nc.scalar.activation(dsb[:, :], bD[:, :], AF.Copy)
                    # keep ACT in step order (scheduler otherwise scrambles)
                    if evac_prev[0] is not None:
                        tile.add_dep_helper(e1.ins, evac_prev[0].ins, False, "ACT order")
                    evac_prev[0] = e1
                    return dsb

                # lookahead-3 software pipeline: PE queue per step is
                # [F(j+3), D(j+3), matvec(j)] so when matvec(j) waits on
                # B(j), the PE has already streamed three steps of outers
                # and (with no gaps) can ramp to its max pstate.
                dsb_q = [emit_outers(0), emit_outers(1), emit_outers(2)]
                for j in range(C):
                    dsb = dsb_q.pop(0)
                    mp = stepp.tile([D, NP * D], BF16, tag="mp")
                    m_new = statep.tile([D, NP * D], BF16, tag="M")
                    nc.vector.scalar_tensor_tensor(
                        out=mp[:, :], in0=bankF[(t0 + j) % 3][:, :], scalar=0.8,
                        in1=m_prev[:, :], op0=AOP.max, op1=AOP.mult,
                    )
                    nc.vector.tensor_add(m_new[:, :], mp[:, :], dsb[:, :])

                    if j + 3 < C:
                        dsb_q.append(emit_outers(j + 3))

                    # masked-Q matvec: one matmul, all pairs
                    nc.tensor.matmul(
                        bankO[:, :],
                        q4r[:, j * D : (j + 1) * D],
                        m_new[:, :],
                        start=(j == 0), stop=(j == C - 1),
                        tile_position=(0, 0),
                    )
                    m_prev = m_new

                nc.scalar.activation(oS[:, :], bankO[:, :], AF.Copy)
                pending_out = (oS, t0)

            if pending_out is not None:
                emit_output(*pending_out)

    nc.compile()
    return nc


_CACHE = {}


def _get_nc(t_run):
    if t_run not in _CACHE:
        _CACHE[t_run] = build(t_run)
    return _CACHE[t_run]


def kernel(q, k, v, f_gate, g_gate):
    t_run = q.shape[1]
    nc = _get_nc(t_run)

    def shard(x):
        # [B, T, H, D] -> [B*H, T, D] -> per-core [NP, T, D]
        xt = np.ascontiguousarray(
            np.transpose(np.asarray(x, dtype=np.float32), (0, 2, 1, 3))
        ).reshape(B * H, t_run, D)
        return [np.ascontiguousarray(xt[c * NP : (c + 1) * NP]) for c in range(N_CORES)]

    qs, ks, vs, fs, gs = (shard(x) for x in (q, k, v, f_gate, g_gate))
    import ml_dtypes
    qts = [
        np.ascontiguousarray(np.transpose(x, (0, 2, 1))).astype(ml_dtypes.bfloat16)
        for x in qs
    ]
    fs = [x.astype(ml_dtypes.bfloat16) for x in fs]
    zeros = np.zeros((C, D), dtype=np.float32)
    in_maps = [
        {"qt": qts[c], "k": ks[c], "v": vs[c], "f": fs[c], "g": gs[c],
         "zeros": zeros}
        for c in range(N_CORES)
    ]
    global _LAST_NC, _LAST_IN_MAPS
    _LAST_NC, _LAST_IN_MAPS = nc, in_maps
    res = run_bass_kernel_spmd(nc, in_maps, core_ids=list(range(N_CORES)))
    full = np.concatenate([res.results[c]["out"] for c in range(N_CORES)], axis=0)
    # [B*H, T, D] -> [B, T, H, D]
    return np.ascontiguousarray(
        np.transpose(full.reshape(B, H, t_run, D), (0, 2, 1, 3))
    )
